# revision 8
# baseline (speedup 1.0000x reference)
"""MoE gating kernel for Trainium2 (Bass/Tile), data-parallel over 8 NeuronCores.

Computes: logits = x @ W_g.T ; top-2 values; softmax over the 2 values.
  p1 = sigmoid(v1 - v2), p2 = sigmoid(v2 - v1)  (v1 >= v2 the top-2 logits)

Sharding: tokens split 8 ways (2048 tokens/core), W_g replicated.

v2 fp16 datapath (vs the f32r v1): the kernel is DMA-roofline bound on the
16.8MB/core HBM read of x (~415 GB/s sustained => ~40us). Everything else is
sized to hide inside that window:
  - x is cast f32->fp16 *during* the SWDGE DMA (gpsimd dma_start casts; read
    side still runs at full HBM rate, SBUF write bytes halve)
  - PE transposes fp16 tiles; with is_transpose the PSUM result stays fp16,
    so the DVE drain runs in 2x_1P packed mode (16-bit) - half the drain
    cycles of the f32 path, plus FWL halves LDWEIGHTS for the transposes
  - logits matmul in fp16 (1 col/cycle, same PE speed as f32r)
  - one-group pipeline skew: per group g the PE runs [transposes(g),
    matmul(g-1), epilogue-transposes(g-2)] so it never waits on DVE drains
  - outputs staged in SBUF, single output DMA at the end
fp16 adds ~1e-3 worst-case abs error on the probabilities (vs 2.4e-4 for
f32r); the harness gate is 2e-2.
"""

import sys

sys.path.insert(0, "/opt/trn_rl_repo")

from contextlib import ExitStack

import numpy as np

import concourse.bass as bass
import concourse.bacc as bacc
import concourse.mybir as mybir
from concourse import masks
from concourse.tile import TileContext
from concourse.bass_utils import run_bass_kernel_spmd

TOKENS = 16384
DIM = 2048
E = 64  # num experts
NCORES = 8
TPC = TOKENS // NCORES  # tokens per core
P = 128
KT = DIM // P  # 16 contraction tiles
G = 256  # token group (one DMA chunk, one matmul moving-dim)
NG = TPC // G  # 8 groups per core
TB = G // P  # 2 token blocks per group

F32 = mybir.dt.float32
F16 = mybir.dt.float16


def _emit(tc: TileContext, ctx: ExitStack, x_ap, wg_ap, out_ap):
    nc = tc.nc

    singles = ctx.enter_context(tc.tile_pool(name="singles", bufs=1))
    xpool = ctx.enter_context(tc.tile_pool(name="xpool", bufs=1))
    xtpool = ctx.enter_context(tc.tile_pool(name="xtpool", bufs=3))
    ltpool = ctx.enter_context(tc.tile_pool(name="ltpool", bufs=2))
    spool = ctx.enter_context(tc.tile_pool(name="spool", bufs=4))
    psum_t = ctx.enter_context(tc.tile_pool(name="psum_t", bufs=4, space="PSUM"))
    psum_l = ctx.enter_context(tc.tile_pool(name="psum_l", bufs=2, space="PSUM"))
    psum_f = ctx.enter_context(tc.tile_pool(name="psum_f", bufs=2, space="PSUM"))

    # --- x streamed in as fp16, one SWDGE cast-DMA per group. 1-tile chunks
    # at both ends: the first shortens the PE ramp-up, the last shortens the
    # serial tail after the final byte lands. Q7 descriptor-gen is ~2.5us per
    # op, under the ~5us of HBM time per 2MB chunk, so it stays ahead.
    TBS = [1, 1, 2, 2, 2, 2, 2, 2, 1, 1]  # tiles per group (sums to 16)
    offs = [sum(TBS[:g]) * P for g in range(len(TBS))]  # token offset per group
    xs = []
    for g, tb in enumerate(TBS):
        xg = xpool.tile([P, tb, DIM], F16, tag=f"x{g}", name=f"x{g}")
        xs.append(xg)

    def load_chunk(g):
        r0 = offs[g]
        nc.gpsimd.dma_start(
            out=xs[g][:],
            in_=x_ap[r0 : r0 + TBS[g] * P, :].rearrange("(t p) d -> p t d", p=P),
        )

    load_chunk(0)
    # W_g [64, 2048] f32 -> fp16 flat [128, 1024]: row (2e+h) = W_g[e, 1024h:+1024]
    wg_sb = singles.tile([P, DIM // 2], F16)
    nc.gpsimd.dma_start(out=wg_sb[:], in_=wg_ap.rearrange("e (h c) -> (e h) c", h=2))
    load_chunk(1)
    load_chunk(2)
    # identities here on the Q7 queue: ready (~12us) before the first
    # transpose needs them, after 3 chunks are queued so the SDMAs never idle
    identh = singles.tile([P, P], F16)
    masks.make_identity(nc, identh[:])
    identf = singles.tile([E, E], F32)
    masks.make_identity(nc, identf[:])
    for g in range(3, len(TBS)):
        load_chunk(g)

    # PE warm-up: dummy matmuls keep the PE busy from engine boot so the HAM
    # clock gate is at 2.4GHz when real transposes start. Also preload the
    # ACT sigmoid table during the DMA wait.
    warm = singles.tile([P, P], F16)
    nc.vector.memset(warm[:], 0.0)
    for _ in range(8):
        pw = psum_f.tile([P, P], F32, tag="fin_ps")
        nc.tensor.matmul(pw[:], warm[:], warm[:])
    sig_warm = spool.tile([P, 2], F32)
    nc.scalar.activation(sig_warm[:], warm[:, 0:2], mybir.ActivationFunctionType.Sigmoid)

    # wgT[c, j, 2e+h] = W_g[e, 1024h + 128j + c]; built after group-0's
    # transposes so the W load never gates PE start.
    wgT = singles.tile([P, KT // 2, P], F16)

    def build_wgT():
        for j in range(KT // 2):
            pt = psum_f.tile([P, P], F16, tag="fin_ps")
            nc.tensor.matmul(
                pt[:], wg_sb[:, j * P : (j + 1) * P], identh[:], is_transpose=True
            )
            nc.vector.tensor_copy(wgT[:, j, :], pt[:])

    def wgT_k(k):
        # [128 d-part, 64 experts] for k-tile k: d = 1024h + 128j + c
        h, j = divmod(k, KT // 2)
        base = wgT[:, j, :]
        return bass.AP(
            tensor=base.tensor,
            offset=base.offset + h,
            ap=[base.ap[0], [2, E]],
        )

    def transposes(g, xt, half):
        # x [t,d] -> xT [128 d, k*(128*tb) t]; 8 fp16 [128,128] transposes fill
        # one PSUM bank ([128,1024] fp16 = 2KB), drained by one 2x-packed DVE
        # copy. Emitted in two halves so mm(g-1) slots between them on the PE.
        xg = xs[g]
        tbs = TBS[g]
        gw = tbs * P  # token width of this group
        kq = 8 // tbs  # k-tiles per PSUM bank
        nq = KT // kq  # PSUM banks for this group
        for q in range(nq // 2 * half, nq // 2 * (half + 1)):
            pt = psum_t.tile([P, 8 * P], F16)
            for dk in range(kq):
                k = q * kq + dk
                for tb in range(tbs):
                    nc.tensor.matmul(
                        pt[:, dk * gw + tb * P : dk * gw + (tb + 1) * P],
                        xg[:, tb, k * P : (k + 1) * P],
                        identh[:],
                        is_transpose=True,
                    )
            nc.vector.tensor_copy(
                xt[:, q * kq * gw : (q + 1) * kq * gw], pt[:]
            )

    def mm_group(g, xt):
        # logitsT [64 e, gw t] = sum_k wgT_k.T @ xT_k
        gw = TBS[g] * P
        lp = psum_l.tile([E, G], F32, name="lp", tag="lp")
        for k in range(KT):
            nc.tensor.matmul(
                lp[:, :gw],
                wgT_k(k),
                xt[:, k * gw : (k + 1) * gw],
                start=(k == 0),
                stop=(k == KT - 1),
            )
        return lp

    # probs staged in SBUF; 8-byte-run output DMAs are deferred past the end
    # of the x stream (concurrent tiny-descriptor DMAs cost the stream ~25%)
    obuf = singles.tile([P, (TPC // P) * 2], F32)

    def epilogue(g, lp):
        # back to token-major + top-2 + softmax into obuf
        gw = TBS[g] * P
        lt = ltpool.tile([E, G], F32, name="lt", tag="lt")
        nc.vector.tensor_copy(lt[:, :gw], lp[:, :gw])
        for tb in range(TBS[g]):
            blk = offs[g] // P + tb
            fp = psum_f.tile([P, E], F32, tag="fin_ps")
            nc.tensor.matmul(
                fp[:], lt[:, tb * P : (tb + 1) * P], identf[:], is_transpose=True
            )
            max8 = spool.tile([P, 8], F32)
            nc.vector.max(out=max8[:], in_=fp[:])
            dd = spool.tile([P, 2], F32)
            nc.vector.tensor_sub(dd[:, 0:1], max8[:, 0:1], max8[:, 1:2])  # v1-v2
            nc.vector.tensor_sub(dd[:, 1:2], max8[:, 1:2], max8[:, 0:1])  # v2-v1
            nc.scalar.activation(
                obuf[:, blk * 2 : (blk + 1) * 2],
                dd[:],
                mybir.ActivationFunctionType.Sigmoid,
            )

    NGV = len(TBS)
    NBLK = TPC // P
    xts, lps = {}, {}
    for g in range(NGV):
        xts[g] = xtpool.tile([P, KT * G], F16, name="xt", tag="xt")
        transposes(g, xts[g], 0)
        if g == 0:
            build_wgT()
        if g >= 1:
            lps[g - 1] = mm_group(g - 1, xts[g - 1])
        if g >= 2:
            epilogue(g - 2, lps.pop(g - 2))
        transposes(g, xts[g], 1)
    lps[NGV - 1] = mm_group(NGV - 1, xts[NGV - 1])
    epilogue(NGV - 2, lps.pop(NGV - 2))
    # blocks up to the second-to-last group: one big DMA issued after the x
    # stream has finished, overlapping the last group's compute
    cut = (offs[NGV - 1] // P + TBS[NGV - 1]) - TBS[NGV - 1]
    nc.sync.dma_start(
        out=out_ap[0 : cut * P, :].rearrange("(b p) c -> p b c", p=P),
        in_=obuf[:, 0 : cut * 2],
    )
    epilogue(NGV - 1, lps.pop(NGV - 1))
    nc.sync.dma_start(
        out=out_ap[cut * P : NBLK * P, :].rearrange("(b p) c -> p b c", p=P),
        in_=obuf[:, cut * 2 : NBLK * 2],
    )


_NC_CACHE = {}


def _build():
    key = "nc"
    if key in _NC_CACHE:
        return _NC_CACHE[key]
    nc = bacc.Bacc(trn_type="TRN2")
    x = nc.dram_tensor("x", [TPC, DIM], F32, kind="ExternalInput")
    wg = nc.dram_tensor("w_g", [E, DIM], F32, kind="ExternalInput")
    out = nc.dram_tensor("out", [TPC, 2], F32, kind="ExternalOutput")
    with TileContext(nc) as tc, ExitStack() as ctx:
        _emit(tc, ctx, x.ap(), wg.ap(), out.ap())
    if not nc.is_finalized():
        nc.finalize()
    _NC_CACHE[key] = nc
    return nc


def _run(x, W_g, trace=False):
    nc = _build()
    x = np.ascontiguousarray(np.asarray(x, dtype=np.float32))
    W_g = np.ascontiguousarray(np.asarray(W_g, dtype=np.float32))
    in_maps = [
        {"x": np.ascontiguousarray(x[c * TPC : (c + 1) * TPC]), "w_g": W_g}
        for c in range(NCORES)
    ]
    res = run_bass_kernel_spmd(nc, in_maps, core_ids=list(range(NCORES)), trace=trace)
    out = np.concatenate([r["out"] for r in res.results], axis=0)
    return out, res


def kernel(x, W_g):
    out, _ = _run(x, W_g, trace=False)
    return out


def kernel_profiled(x, W_g, mm_f32r=True):
    # mm_f32r kept for test.py compatibility; the v2 kernel is fp16-only
    out, res = _run(x, W_g, trace=True)
    return out, res
